# revision 2
# baseline (speedup 1.0000x reference)
"""Trainium2 Bass kernel for nn_CombinedGraphLayer (LSH-binned GNN message passing).

Full inputs in, full output out. Shards batch (B=16) over 8 NeuronCores (2 per core).
Per batch on device:
  A) ffn_dist (fp32 exact on the LSH-critical path), x_dist/na rows, rot `mul`
  B) argmax -> bin index per point
  C) counting-sort ranks (one-hot + triangular matmuls, exact integer arithmetic)
  D) scatter x_dist rows / x(bf16) rows to slot-major scratch via ranks
  E) per-bin pairwise gaussian adj + 2x GHConv (bf16 matmuls)
  F) gather slot-major results back to point order
"""
import sys
sys.path.insert(0, "/opt/trn_rl_repo")
import numpy as np
import ml_dtypes

import concourse.bass as bass
import concourse.bacc as bacc
import concourse.mybir as mybir
from concourse.tile import TileContext
from concourse.bass_utils import run_bass_kernel_spmd

F32 = mybir.dt.float32
BF16 = mybir.dt.bfloat16
I32 = mybir.dt.int32
U8 = mybir.dt.uint8
AF = mybir.ActivationFunctionType
OP = mybir.AluOpType

B, N, F = 16, 12800, 256
BIN = 128
NB = N // BIN          # 100 bins
NCORES = 8
BPC = B // NCORES      # 2 batches per core
NT = N // 128          # 100 point tiles
BIG = 1000.0
V = 200                # one-hot width (bin ids 0..198)

bf = ml_dtypes.bfloat16


def build_batch(nc, tc, ct, xT_d, xb_d, msk_d, out_d, xd_pm, xd_sm, xb_sm, out_sm, gg_dr, dbg=None):
    W1, B1, W2, B2M, WR, BRR = ct["W1"], ct["B1"], ct["W2"], ct["B2M"], ct["WR"], ct["BRR"]
    ACST, BCST, IOTAV = ct["ACST"], ct["BCST"], ct["IOTAV"]
    UT128, UTREP = ct["UT128"], ct["UTREP"]
    ONES1_BF, ONES1_F, ONESC_BF, ONES100_F = ct["ONES1_BF"], ct["ONES1_F"], ct["ONESC_BF"], ct["ONES100_F"]
    IDENT, IDENTB, ID100, E6 = ct["IDENT"], ct["IDENTB"], ct["ID100"], ct["E6"]
    IDENTB2 = ct["IDENTB2"]
    WCV, BT = ct["WCV"], ct["BT"]

    import contextlib
    with tc.tile_pool(name="big", bufs=1) as bigp, \
         tc.tile_pool(name="wrk", bufs=2) as wp:

        mul_all = bigp.tile([128, NT, 50], F32, tag="mul_all")
        bins_bf = bigp.tile([128, NT], BF16, tag="bins_bf")
        ranks_f = bigp.tile([128, NT], F32, tag="ranks_f")
        ranks_i = bigp.tile([128, NT], I32, tag="ranks_i")
        O_all = bigp.tile([128, NT, V], BF16, tag="O_all")
        msk_f = bigp.tile([128, NT], F32, tag="msk_f")

        msk_u8 = wp.tile([128, NT], U8)
        nc.sync.dma_start(out=msk_u8[:], in_=msk_d.rearrange("(t p) -> p t", p=128))
        nc.vector.tensor_copy(out=msk_f[:], in_=msk_u8[:])

        # ---------------- Phase A: ffn + rot ----------------
        GPT = 4
        ppA = tc.tile_pool(name="psA", bufs=2, space="PSUM")
        pp = ppA.__enter__()
        for g in range(NT // GPT):
            xT_t = wp.tile([128, 2, GPT * 128], F32, tag="xT")
            nc.sync.dma_start(
                out=xT_t[:],
                in_=xT_d[:, g * GPT * 128:(g + 1) * GPT * 128].rearrange("(c p) n -> p c n", p=128))
            ph1 = [pp.tile([128, 512], F32, tag=f"ph1{h}", name=f"ph1{h}") for h in range(2)]
            for h in range(2):
                for k in range(2):
                    nc.tensor.matmul(out=ph1[h][:], lhsT=W1[:, k, h * 128:(h + 1) * 128],
                                     rhs=xT_t[:, k, :], start=(k == 0), stop=(k == 1))
            h1f = wp.tile([128, 2, 512], F32, tag="h1f")
            h1b = wp.tile([128, 2, 512], BF16, tag="h1b")
            for h in range(2):
                mn = wp.tile([128, 512], F32, tag="mn")
                nc.vector.tensor_scalar(out=mn[:], in0=ph1[h][:], scalar1=B1[:, h:h + 1],
                                        scalar2=0.0, op0=OP.add, op1=OP.min)
                ee = wp.tile([128, 512], F32, tag="ee")
                nc.scalar.activation(out=ee[:], in_=mn[:], func=AF.Exp)
                rr = wp.tile([128, 512], F32, tag="rr")
                nc.scalar.activation(out=rr[:], in_=ph1[h][:], func=AF.Relu, bias=B1[:, h:h + 1])
                nc.vector.tensor_tensor(out=h1f[:, h, :], in0=rr[:], in1=ee[:], op=OP.add)
                nc.vector.tensor_copy(out=h1b[:, h, :], in_=h1f[:, h, :])
            for p4 in range(GPT):
                t = g * GPT + p4
                sl = slice(p4 * 128, (p4 + 1) * 128)
                pxd = pp.tile([128, 512], F32, tag="pxd")
                for k in range(2):
                    nc.tensor.matmul(out=pxd[:, 0:128], lhsT=h1b[:, k, sl], rhs=W2[:, k, :],
                                     start=(k == 0), stop=(k == 1))
                nc.tensor.matmul(out=pxd[:, 128:178], lhsT=ONES1_F[:], rhs=BRR[:],
                                 start=True, stop=False)
                for k in range(2):
                    nc.tensor.matmul(out=pxd[:, 128:178], lhsT=h1f[:, k, sl], rhs=WR[:, k, :],
                                     start=False, stop=(k == 1))
                rowt = wp.tile([128, 132], F32, tag="rowt")
                nc.vector.tensor_tensor(out=rowt[:, 0:128], in0=pxd[:, 0:128], in1=B2M[:],
                                        op=OP.add)
                sq = wp.tile([128, 128], BF16, tag="sq")
                nc.scalar.activation(out=sq[:], in_=rowt[:, 0:128], func=AF.Square,
                                     accum_out=rowt[:, 128:129])
                nc.vector.tensor_copy(out=rowt[:, 129:130], in_=msk_f[:, t:t + 1])
                nc.vector.memset(rowt[:, 130:132], 0.0)
                nc.vector.tensor_copy(out=mul_all[:, t, :], in_=pxd[:, 128:178])
                nc.sync.dma_start(out=xd_pm[t * 128:(t + 1) * 128, :], in_=rowt[:])
                if dbg is not None and t == 0:
                    nc.sync.dma_start(out=dbg[2][:, :], in_=rowt[:])

        ppA.__exit__(None, None, None)
        # ---------------- Phase B: argmax -> bins ----------------
        mx = wp.tile([128, NT], F32, tag="mx")
        nc.vector.tensor_reduce(out=mx[:], in_=mul_all[:], axis=mybir.AxisListType.X,
                                op=OP.max, apply_absolute_value=True)
        nmx = wp.tile([128, NT], F32, tag="nmx")
        nc.vector.tensor_scalar_mul(nmx[:], mx[:], -1.0)
        eqp = bigp.tile([128, NT, 50], F32, tag="O_all", name="eqp")
        eqn = mul_all
        nc.vector.tensor_tensor(out=eqp[:], in0=mul_all[:],
                                in1=mx[:].unsqueeze(2).to_broadcast([128, NT, 50]), op=OP.is_equal)
        nc.vector.tensor_tensor(out=eqn[:], in0=mul_all[:],
                                in1=nmx[:].unsqueeze(2).to_broadcast([128, NT, 50]), op=OP.is_equal)
        nc.vector.tensor_tensor(out=eqp[:], in0=eqp[:],
                                in1=ACST[:].unsqueeze(1).to_broadcast([128, NT, 50]), op=OP.mult)
        nc.vector.tensor_tensor(out=eqn[:], in0=eqn[:],
                                in1=BCST[:].unsqueeze(1).to_broadcast([128, NT, 50]), op=OP.mult)
        nc.vector.tensor_tensor(out=eqp[:], in0=eqp[:], in1=eqn[:], op=OP.max)
        rmx = wp.tile([128, NT], F32, tag="rmx")
        nc.vector.tensor_reduce(out=rmx[:], in_=eqp[:], axis=mybir.AxisListType.X, op=OP.max)
        m99 = wp.tile([128, NT], F32, tag="m99")
        nc.vector.tensor_scalar(out=m99[:], in0=msk_f[:], scalar1=-99.0, scalar2=99.0,
                                op0=OP.mult, op1=OP.add)
        binsf = wp.tile([128, NT], F32, tag="binsf")
        nc.vector.tensor_scalar(out=binsf[:], in0=rmx[:], scalar1=-1.0, scalar2=BIG,
                                op0=OP.mult, op1=OP.add)
        nc.vector.tensor_tensor(out=binsf[:], in0=binsf[:], in1=m99[:], op=OP.add)
        nc.vector.tensor_copy(out=bins_bf[:], in_=binsf[:])
        if dbg is not None:
            nc.sync.dma_start(out=dbg[0][:, :], in_=binsf[:])

        # ---------------- Phase C: ranks ----------------
        for t in range(NT):
            nc.vector.tensor_tensor(out=O_all[:, t, :], in0=IOTAV[:],
                                    in1=bins_bf[:, t:t + 1].to_broadcast([128, V]),
                                    op=OP.is_equal)
        ppC = tc.tile_pool(name="psC", bufs=1, space="PSUM")
        pp1 = ppC.__enter__()
        ppC2 = tc.tile_pool(name="psC2", bufs=2, space="PSUM")
        pp = ppC2.__enter__()
        pG = pp1.tile([100, 512], F32, tag="pG")
        ph99 = pp1.tile([1, 512], F32, tag="ph99")
        for t in range(NT):
            nc.tensor.matmul(out=pG[:, 0:V], lhsT=UTREP[:, t, :], rhs=O_all[:, t, :],
                             start=(t == 0), stop=(t == NT - 1))
        for t in range(NT):
            nc.tensor.matmul(out=ph99[:, 0:V], lhsT=ONESC_BF[:], rhs=O_all[:, t, :],
                             start=(t == 0), stop=(t == NT - 1))
        Gpre = wp.tile([100, V], F32, tag="Gpre")
        nc.vector.tensor_copy(out=Gpre[:], in_=pG[:, 0:V])
        htot = wp.tile([1, V], F32, tag="htot")
        nc.vector.tensor_copy(out=htot[:], in_=ph99[:, 0:V])
        wz = wp.tile([1, V], F32, tag="wz")
        nc.vector.memset(wz[:], 0.0)
        incl = wp.tile([1, V], F32, tag="incl")
        nc.vector.tensor_tensor_scan(out=incl[:], data0=htot[:], data1=wz[:],
                                     initial=0.0, op0=OP.add, op1=OP.add)
        off = wp.tile([1, V], F32, tag="off")
        nc.vector.memset(off[:, 0:1], 0.0)
        nc.vector.tensor_copy(out=off[:, 1:V], in_=incl[:, 0:V - 1])
        pG2 = pp1.tile([100, 512], F32, tag="pG2")
        nc.tensor.matmul(out=pG2[:, 0:V], lhsT=ONES100_F[:], rhs=off[:], start=True, stop=False)
        nc.tensor.matmul(out=pG2[:, 0:V], lhsT=ID100[:], rhs=Gpre[:], start=False, stop=True)
        gi = wp.tile([100, V], I32, tag="gi")
        nc.vector.tensor_copy(out=gi[:], in_=pG2[:, 0:V])
        glo = wp.tile([100, V], I32, tag="glo")
        nc.vector.tensor_scalar(out=glo[:], in0=gi[:], scalar1=127, scalar2=None,
                                op0=OP.bitwise_and)
        ghi = wp.tile([100, V], I32, tag="ghi")
        nc.vector.tensor_scalar(out=ghi[:], in0=gi[:], scalar1=-128, scalar2=None,
                                op0=OP.bitwise_and)
        GG = wp.tile([100, 2 * V], BF16, tag="GG")
        nc.vector.tensor_copy(out=GG[:, 0:V], in_=glo[:])
        nc.vector.tensor_copy(out=GG[:, V:2 * V], in_=ghi[:])
        nc.sync.dma_start(out=gg_dr[:, :], in_=GG[:])

        # pass 2 (quartered GG to keep partition-0 footprint small)
        scr = wp.tile([128, 2 * V], BF16, tag="scr")
        for q in range(4):
            GG1 = bigp.tile([1, 25, 2 * V], BF16, tag="mul_all", name="GG1")
            nc.sync.dma_start(out=GG1[:], in_=gg_dr[q * 25:(q + 1) * 25, :].rearrange("t v -> (t v)").unsqueeze(0).rearrange("o (t v) -> o t v", t=25))
            for tq in range(25):
                t = q * 25 + tq
                pS = pp.tile([128, 512], F32, tag="pS")
                nc.tensor.matmul(out=pS[:, 0:2 * V], lhsT=ONES1_BF[:], rhs=GG1[:, tq, :],
                                 start=True, stop=False)
                nc.tensor.matmul(out=pS[:, 0:V], lhsT=UT128[:], rhs=O_all[:, t, :],
                                 start=False, stop=True)
                nc.vector.scalar_tensor_tensor(
                    out=scr[:], in0=pS[:, 0:2 * V].rearrange("p (c v) -> p c v", c=2), scalar=1.0,
                    in1=O_all[:, t, :].unsqueeze(1).to_broadcast([128, 2, V]),
                    op0=OP.mult, op1=OP.mult, accum_out=ranks_f[:, t:t + 1])
        nc.vector.tensor_copy(out=ranks_i[:], in_=ranks_f[:])
        if dbg is not None:
            nc.sync.dma_start(out=dbg[1][:, :], in_=ranks_f[:])
        ppC2.__exit__(None, None, None)
        ppC.__exit__(None, None, None)

        # ---------------- Phase D: scatter to slot-major ----------------
        for t in range(NT):
            xdt = wp.tile([128, 132], F32, tag="xdt")
            nc.sync.dma_start(out=xdt[:], in_=xd_pm[t * 128:(t + 1) * 128, :])
            nc.gpsimd.indirect_dma_start(
                out=xd_sm[:, :], out_offset=bass.IndirectOffsetOnAxis(ap=ranks_i[:, t:t + 1], axis=0),
                in_=xdt[:], in_offset=None)
            xbt = wp.tile([128, 256], BF16, tag="xbt")
            nc.sync.dma_start(out=xbt[:], in_=xb_d[t * 128:(t + 1) * 128, :])
            nc.gpsimd.indirect_dma_start(
                out=xb_sm[:, :], out_offset=bass.IndirectOffsetOnAxis(ap=ranks_i[:, t:t + 1], axis=0),
                in_=xbt[:], in_offset=None)

        # ---------------- Phase E: conv per bin ----------------
        CG = 4
        ppE = tc.tile_pool(name="psE", bufs=2, space="PSUM")
        pp = ppE.__enter__()
        for g in range(NB // CG):
            dm_l, n_l, xds_l, xbs_l = [], [], [], []
            idg = wp.tile([128, CG], F32, tag="idg")
            for j in range(CG):
                c = g * CG + j
                xds = wp.tile([128, 132], F32, tag=f"xds{j}")
                nc.sync.dma_start(out=xds[:], in_=xd_sm[c * 128:(c + 1) * 128, :])
                xbs = wp.tile([128, 256], BF16, tag=f"xbs{j}")
                nc.sync.dma_start(out=xbs[:], in_=xb_sm[c * 128:(c + 1) * 128, :])
                xds_l.append(xds)
                xbs_l.append(xbs)
                pC = pp.tile([128, 512], F32, tag="pC")
                nc.tensor.transpose(out=pC[:, 0:128], in_=xds[:, 0:128], identity=IDENT[:])
                xmT = wp.tile([128, 128], F32, tag="xmT")
                nc.vector.tensor_copy(out=xmT[:], in_=pC[:, 0:128])
                xmT2 = wp.tile([128, 128], F32, tag="xmT2")
                nc.vector.tensor_scalar_mul(xmT2[:], pC[:, 0:128], -2.0)
                pNA = pp.tile([1, 128], F32, tag="pZH", name="pNA")
                nc.tensor.matmul(out=pC[:, 128:256], lhsT=xmT2[:], rhs=xmT[:], start=True, stop=False)
                nc.tensor.transpose(out=pNA[:], in_=xds[:, 128:129], identity=IDENT[:])
                naR = wp.tile([1, 128], F32, tag="naR")
                nc.vector.tensor_copy(out=naR[:], in_=pNA[:])
                nc.tensor.matmul(out=pC[:, 128:256], lhsT=ONES1_F[:], rhs=naR[:], start=False, stop=True)
                D2 = wp.tile([128, 128], F32, tag="D2")
                nc.vector.scalar_tensor_tensor(out=D2[:], in0=pC[:, 128:256], scalar=xds[:, 128:129],
                                               in1=E6[:].to_broadcast([128, 128]), op0=OP.add, op1=OP.max)
                if dbg is not None and c == 0:
                    nc.sync.dma_start(out=dbg[7][:, :], in_=D2[:])
                    nc.sync.dma_start(out=dbg[8][:, :], in_=naR[:])
                nc.scalar.activation(out=D2[:], in_=D2[:], func=AF.Ln)
                nc.scalar.activation(out=D2[:], in_=D2[:], func=AF.Exp, scale=0.5)
                dm = wp.tile([128, 128], BF16, tag=f"dm{j}")
                nc.scalar.activation(out=dm[:], in_=D2[:], func=AF.Exp, scale=-0.1)
                dm_l.append(dm)
                mskb = wp.tile([128, 1], BF16, tag="mskb")
                nc.vector.tensor_copy(out=mskb[:], in_=xds[:, 129:130])
                nc.tensor.matmul(out=pC[:, 384:385], lhsT=dm[:], rhs=mskb[:], start=True, stop=True)
                nc.vector.scalar_tensor_tensor(out=idg[:, j:j + 1], in0=pC[:, 384:385],
                                               scalar=xds[:, 129:130],
                                               in1=E6[:], op0=OP.mult, op1=OP.add)
                if dbg is not None and c == 0:
                    nc.sync.dma_start(out=dbg[3][:, :], in_=xds[:])
                    nc.sync.dma_start(out=dbg[4][:, :], in_=dm[:])
            nc.scalar.activation(out=idg[:], in_=idg[:], func=AF.Ln)
            nc.scalar.activation(out=idg[:], in_=idg[:], func=AF.Exp, scale=-0.5)
            for j in range(CG):
                nco = wp.tile([128, 1], F32, tag=f"nco{j}")
                nc.vector.tensor_tensor(out=nco[:], in0=idg[:, j:j + 1], in1=xds_l[j][:, 129:130],
                                        op=OP.mult)
                n_l.append(nco)
                if dbg is not None and g == 0 and j == 0:
                    nc.sync.dma_start(out=dbg[5][:, :], in_=nco[:])
            for j in range(CG):
                c = g * CG + j
                xds, dm, nco = xds_l[j], dm_l[j], n_l[j]
                cur = wp.tile([128, 256], BF16, tag="cur")
                nc.vector.tensor_scalar_mul(cur[:], xbs_l[j][:], xds[:, 129:130])
                for l in range(2):
                    pT = pp.tile([128, 256], BF16, tag="pTB")
                    xbT = wp.tile([128, 2, 128], BF16, tag="xbT")
                    for k in range(2):
                        nc.tensor.transpose(out=pT[:, k * 128:(k + 1) * 128],
                                            in_=cur[:, k * 128:(k + 1) * 128], identity=IDENTB[:])
                        nc.vector.tensor_copy(out=xbT[:, k, :], in_=pT[:, k * 128:(k + 1) * 128])
                    pZH = pp.tile([128, 512], F32, tag="pZH")
                    pPH = pp.tile([128, 512], F32, tag="pPH")
                    nc.tensor.matmul(out=pZH[:, 0:256], lhsT=ONES1_BF[:], rhs=BT[:, l, :],
                                     start=True, stop=False)
                    for k in range(2):
                        nc.tensor.matmul(out=pZH[:, 0:256], lhsT=xbT[:, k, :],
                                         rhs=WCV[:, k, 6 * l + 0, :], start=False, stop=(k == 1))
                    for k in range(2):
                        nc.tensor.matmul(out=pZH[:, 256:512], lhsT=xbT[:, k, :],
                                         rhs=WCV[:, k, 6 * l + 2, :], start=(k == 0), stop=(k == 1))
                    for k in range(2):
                        nc.tensor.matmul(out=pPH[:, 0:256], lhsT=xbT[:, k, :],
                                         rhs=WCV[:, k, 6 * l + 4, :], start=(k == 0), stop=(k == 1))
                    tg = wp.tile([128, 256], BF16, tag="tg")
                    nc.scalar.activation(out=tg[:], in_=pZH[:, 0:256], func=AF.Tanh, scale=0.5)
                    eh = wp.tile([128, 256], BF16, tag="eh")
                    nc.scalar.activation(out=eh[:], in_=pZH[:, 256:512], func=AF.Copy)
                    h2 = wp.tile([128, 256], BF16, tag="h2")
                    nc.scalar.activation(out=h2[:], in_=pPH[:, 0:256], func=AF.Copy, scale=nco[:])
                    nc.tensor.matmul(out=pPH[:, 256:512], lhsT=dm[:], rhs=h2[:], start=True, stop=True)
                    dd = wp.tile([128, 256], BF16, tag="dd")
                    nc.vector.scalar_tensor_tensor(out=dd[:], in0=pPH[:, 256:512], scalar=nco[:],
                                                   in1=eh[:], op0=OP.mult, op1=OP.subtract)
                    uu = wp.tile([128, 256], BF16, tag="uu")
                    nc.vector.tensor_tensor(out=uu[:], in0=tg[:], in1=dd[:], op=OP.mult)
                    nc.vector.tensor_tensor(out=uu[:], in0=uu[:], in1=dd[:], op=OP.add)
                    ob = wp.tile([128, 256], BF16, tag="ob")
                    nc.vector.scalar_tensor_tensor(out=ob[:], in0=uu[:], scalar=0.5, in1=eh[:],
                                                   op0=OP.mult, op1=OP.add)
                    mn2 = wp.tile([128, 256], BF16, tag="mn2")
                    nc.vector.tensor_scalar_min(mn2[:], ob[:], 0.0)
                    ex2 = wp.tile([128, 256], BF16, tag="ex2")
                    nc.scalar.activation(out=ex2[:], in_=mn2[:], func=AF.Exp)
                    rr2 = wp.tile([128, 256], BF16, tag="rr2")
                    nc.vector.scalar_tensor_tensor(out=rr2[:], in0=ob[:], scalar=0.0, in1=ex2[:],
                                                   op0=OP.max, op1=OP.add)
                    if l == 0:
                        nxt = wp.tile([128, 256], BF16, tag="cur")
                        nc.vector.scalar_tensor_tensor(
                            out=nxt[:], in0=rr2[:], scalar=-1.0,
                            in1=xds[:, 129:130].to_broadcast([128, 256]), op0=OP.add, op1=OP.mult)
                        if dbg is not None and c == 0:
                            nc.sync.dma_start(out=dbg[6][:, :], in_=nxt[:])
                        cur = nxt
                    else:
                        fin = wp.tile([128, 256], F32, tag="fin")
                        nc.vector.scalar_tensor_tensor(
                            out=fin[:], in0=rr2[:], scalar=-1.0,
                            in1=xds[:, 129:130].to_broadcast([128, 256]), op0=OP.add, op1=OP.mult)
                        nc.sync.dma_start(out=out_sm[c * 128:(c + 1) * 128, :], in_=fin[:])

        ppE.__exit__(None, None, None)
        # ---------------- Phase F: gather back to point order ----------------
        for t in range(NT):
            og = wp.tile([128, 256], F32, tag="og")
            nc.gpsimd.indirect_dma_start(
                out=og[:], out_offset=None, in_=out_sm[:, :],
                in_offset=bass.IndirectOffsetOnAxis(ap=ranks_i[:, t:t + 1], axis=0))
            nc.sync.dma_start(out=out_d[t * 128:(t + 1) * 128, :], in_=og[:])


def build_nc(consts):
    nc = bacc.Bacc(None, target_bir_lowering=False)
    xT_d = [nc.dram_tensor(f"xT{b}", [F, N], F32, kind="ExternalInput") for b in range(BPC)]
    xb_d = [nc.dram_tensor(f"xb{b}", [N, F], BF16, kind="ExternalInput") for b in range(BPC)]
    msk_d = [nc.dram_tensor(f"msk{b}", [N], U8, kind="ExternalInput") for b in range(BPC)]
    out_d = [nc.dram_tensor(f"out{b}", [N, F], F32, kind="ExternalOutput") for b in range(BPC)]
    xd_pm = [nc.dram_tensor(f"xdpm{b}", [N, 132], F32) for b in range(BPC)]
    xd_sm = [nc.dram_tensor(f"xdsm{b}", [N, 132], F32) for b in range(BPC)]
    xb_sm = [nc.dram_tensor(f"xbsm{b}", [N, F], BF16) for b in range(BPC)]
    out_sm = [nc.dram_tensor(f"outsm{b}", [N, F], F32) for b in range(BPC)]
    gg_dr = [nc.dram_tensor(f"ggdr{b}", [100, 2 * V], BF16) for b in range(BPC)]
    dbg = [nc.dram_tensor("dbg0", [128, NT], F32, kind="ExternalOutput"),
           nc.dram_tensor("dbg1", [128, NT], F32, kind="ExternalOutput"),
           nc.dram_tensor("dbg2", [128, 132], F32, kind="ExternalOutput"),
           nc.dram_tensor("dbg3", [128, 132], F32, kind="ExternalOutput"),
           nc.dram_tensor("dbg4", [128, 128], BF16, kind="ExternalOutput"),
           nc.dram_tensor("dbg5", [128, 1], F32, kind="ExternalOutput"),
           nc.dram_tensor("dbg6", [128, 256], BF16, kind="ExternalOutput"),
           nc.dram_tensor("dbg7", [128, 128], F32, kind="ExternalOutput"),
           nc.dram_tensor("dbg8", [1, 128], F32, kind="ExternalOutput")]

    cd = {k: nc.inline_tensor(v, name=k) for k, v in consts.items()}
    with TileContext(nc) as tc:
        with tc.tile_pool(name="const", bufs=1) as cp:
            ct = {}
            for k, v in consts.items():
                t = cp.tile(list(v.shape), mybir.dt.from_np(v.dtype), tag=k, name=k)
                nc.sync.dma_start(out=t[:], in_=cd[k][:])
                ct[k] = t
            for b in range(BPC):
                build_batch(nc, tc, ct, xT_d[b], xb_d[b], msk_d[b], out_d[b],
                            xd_pm[b], xd_sm[b], xb_sm[b], out_sm[b], gg_dr[b],
                            dbg=(dbg if b == 0 else None))
    nc.finalize()
    return nc


def make_consts(w_d1, b_d1, w_d2, b_d2, rot, wt0, bt0, wh0, th0, wt1, bt1, wh1, th1):
    rot50 = rot[:, :NB // 2]
    WR = (w_d2 @ rot50).astype(np.float32)
    # phase A produces h1' = elu+1; fold the -1 into downstream biases
    b2_eff = (b_d2 - w_d2.sum(0)).astype(np.float32)
    BRR = (b2_eff @ rot50).astype(np.float32)[None]

    jj = np.arange(50, dtype=np.float32)
    ACST = (BIG - jj)[None].repeat(128, 0).astype(np.float32)
    BCST = (BIG - 50.0 - jj)[None].repeat(128, 0).astype(np.float32)
    IOTAV = np.arange(V, dtype=np.float32)[None].repeat(128, 0).astype(bf)
    UT128 = (np.arange(128)[:, None] < np.arange(128)[None, :]).astype(bf)
    ut = (np.arange(NB)[:, None] < np.arange(NB)[None, :]).astype(bf)  # [t, tt]=1 iff t<tt
    UTREP = np.broadcast_to(ut[None], (128, NB, NB)).copy()

    W1 = w_d1.reshape(2, 128, 256).transpose(1, 0, 2).astype(np.float32).copy()
    B1 = b_d1.reshape(2, 128).T.astype(np.float32).copy()
    W2 = w_d2.reshape(2, 128, 128).transpose(1, 0, 2).astype(bf).copy()
    B2M = b2_eff[None].repeat(128, 0).astype(np.float32)
    WRc = WR.reshape(2, 128, 50).transpose(1, 0, 2).astype(np.float32).copy()

    wcv = np.zeros((128, 2, 12, 256), dtype=bf)
    for l, (wt, wh, th) in enumerate([(wt0, wh0, th0), (wt1, wh1, th1)]):
        for i, w in enumerate([wt, wh, th]):
            wc = w.reshape(2, 128, 256)
            wcv[:, 0, 6 * l + 2 * i, :] = wc[0].astype(bf)
            wcv[:, 1, 6 * l + 2 * i, :] = wc[1].astype(bf)
    BT = np.stack([bt0, bt1])[None].astype(bf)

    return {
        "W1": W1, "B1": B1, "W2": W2, "B2M": B2M, "WR": WRc, "BRR": BRR,
        "ACST": ACST, "BCST": BCST, "IOTAV": IOTAV, "UT128": UT128, "UTREP": UTREP,
        "ONES1_BF": np.ones((1, 128), dtype=bf),
        "ONES1_F": np.ones((1, 128), dtype=np.float32),
        "ONESC_BF": np.ones((128, 1), dtype=bf),
        "ONES100_F": np.ones((1, 100), dtype=np.float32),
        "IDENT": np.eye(128, dtype=np.float32),
        "IDENTB": np.eye(128, dtype=bf),
        "IDENTB2": (-2.0 * np.eye(128)).astype(bf),
        "ID100": np.eye(100, dtype=np.float32),
        "E6": np.full((128, 1), 1e-6, dtype=np.float32),
        "WCV": wcv, "BT": BT,
    }


_cache = {}


def kernel(x, msk, rot, w_d1, b_d1, w_d2, b_d2, wt0, bt0, wh0, th0, wt1, bt1, wh1, th1):
    x = np.asarray(x, np.float32)
    msk_u8 = np.asarray(msk).astype(np.uint8)
    consts = make_consts(np.asarray(w_d1, np.float32), np.asarray(b_d1, np.float32),
                         np.asarray(w_d2, np.float32), np.asarray(b_d2, np.float32),
                         np.asarray(rot, np.float32),
                         np.asarray(wt0, np.float32), np.asarray(bt0, np.float32),
                         np.asarray(wh0, np.float32), np.asarray(th0, np.float32),
                         np.asarray(wt1, np.float32), np.asarray(bt1, np.float32),
                         np.asarray(wh1, np.float32), np.asarray(th1, np.float32))
    if "nc" not in _cache:
        _cache["nc"] = build_nc(consts)
    nc = _cache["nc"]

    xT = np.ascontiguousarray(x.transpose(0, 2, 1))
    xb16 = np.ascontiguousarray(x.astype(bf))

    in_maps = []
    for c in range(NCORES):
        m = {}
        for b in range(BPC):
            gb = c * BPC + b
            m[f"xT{b}"] = xT[gb]
            m[f"xb{b}"] = xb16[gb]
            m[f"msk{b}"] = msk_u8[gb]
        in_maps.append(m)

    import os as _os
    _trace = _os.environ.get("KERNEL_TRACE", "") == "1"
    res = run_bass_kernel_spmd(nc, in_maps, core_ids=list(range(NCORES)), trace=_trace)
    if _trace:
        print(f"HW exec time: {res.exec_time_ns} ns")
        _cache["exec_time_ns"] = res.exec_time_ns
        _cache["res"] = res
    _cache["dbg"] = {k: res.results[0][k] for k in ("dbg0", "dbg1", "dbg2", "dbg3", "dbg4", "dbg5", "dbg6", "dbg7", "dbg8")}
    out = np.empty((B, N, F), np.float32)
    for c in range(NCORES):
        for b in range(BPC):
            out[c * BPC + b] = res.results[c][f"out{b}"]
    return out



# revision 5
# speedup vs baseline: 1.1492x; 1.1492x over previous
"""Trainium2 Bass kernel for nn_CombinedGraphLayer (LSH-binned GNN message passing).

Full inputs in, full output out. Shards batch (B=16) over 8 NeuronCores (2 per core).
Per batch on device:
  A) ffn_dist (fp32 on the LSH-critical path), writes xd rows (xd|na|msk) to DRAM,
     keeps rot `mul` in SBUF
  B) argmax -> bin index per point
  C) counting-sort ranks (exact integer arithmetic via one-hot + triangular matmuls);
     iota-scatter by rank builds the inverse permutation inv (slot -> point)
  E0) per-bin: indirect-gather xd rows by inv, pairwise gaussian adjacency (bf16
      matmul, consistent na), degree + norm.  Only Ln/Exp activations.
  E12) per-bin: indirect-gather x rows by inv, two fused GHConv layers (bf16),
      tanh gate + exp elu (same act table set), final rows indirect-scattered
      straight to the output by inv.  No slot-major DRAM round trips.
"""
import sys
sys.path.insert(0, "/opt/trn_rl_repo")
import numpy as np
import ml_dtypes

import concourse.bass as bass
import concourse.bacc as bacc
import concourse.mybir as mybir
from concourse.tile import TileContext
from concourse.bass_utils import run_bass_kernel_spmd

F32 = mybir.dt.float32
BF16 = mybir.dt.bfloat16
I32 = mybir.dt.int32
U8 = mybir.dt.uint8
AF = mybir.ActivationFunctionType
OP = mybir.AluOpType

B, N, F = 16, 12800, 256
BIN = 128
NB = N // BIN          # 100 bins
NCORES = 8
BPC = B // NCORES      # 2 batches per core
NT = N // 128          # 100 point tiles
BIG = 1000.0
V = 200                # one-hot width (bin ids 0..198)

bf = ml_dtypes.bfloat16


def build_batch(nc, tc, ct, xT_d, xb_d, msk_d, out_d, xd_pm, inv_dr, gg_dr):
    W1, B1, W2, B2M, WR, BRR = ct["W1"], ct["B1"], ct["W2"], ct["B2M"], ct["WR"], ct["BRR"]
    ACST, BCST, IOTAV = ct["ACST"], ct["BCST"], ct["IOTAV"]
    UT128, UTREP = ct["UT128"], ct["UTREP"]
    ONES1_BF, ONES1_F, ONESC_BF, ONES100_F = ct["ONES1_BF"], ct["ONES1_F"], ct["ONESC_BF"], ct["ONES100_F"]
    IDENT, IDENTB, ID100, E6 = ct["IDENT"], ct["IDENTB"], ct["ID100"], ct["E6"]
    WCV, BTL, IOTA32 = ct["WCV"], ct["BTL"], ct["IOTA32"]

    with tc.tile_pool(name="big", bufs=1) as bigp, \
         tc.tile_pool(name="wrk", bufs=2) as wp:

        mul_all = bigp.tile([128, NT, 50], F32, tag="mul_all")
        bins_bf = bigp.tile([128, NT], BF16, tag="bins_bf")
        ranks_f = bigp.tile([128, NT], F32, tag="ranks_f")
        ranks_i = bigp.tile([128, NT], I32, tag="ranks_i")
        O_all = bigp.tile([128, NT, V], BF16, tag="O_all")
        msk_f = bigp.tile([128, NT], F32, tag="msk_f")
        dm_all = bigp.tile([128, NB, 128], BF16, tag="dm_all")
        nco_all = bigp.tile([128, NB], F32, tag="nco_all")
        msk_all = bigp.tile([128, NB], F32, tag="msk_all")
        mh_all = bigp.tile([128, NB], F32, tag="mh_all")
        inv_sm = bigp.tile([128, NB], I32, tag="inv_sm")

        msk_u8 = wp.tile([128, NT], U8)
        nc.sync.dma_start(out=msk_u8[:], in_=msk_d.rearrange("(t p) -> p t", p=128))
        nc.vector.tensor_copy(out=msk_f[:], in_=msk_u8[:])

        # ---------------- Phase A: ffn + rot ----------------
        GPT = 4
        ppA = tc.tile_pool(name="psA", bufs=2, space="PSUM")
        pp = ppA.__enter__()
        for g in range(NT // GPT):
            xT_t = wp.tile([128, 2, GPT * 128], F32, tag="xT")
            nc.sync.dma_start(
                out=xT_t[:],
                in_=xT_d[:, g * GPT * 128:(g + 1) * GPT * 128].rearrange("(c p) n -> p c n", p=128))
            ph1 = [pp.tile([128, 512], F32, tag=f"ph1{h}", name=f"ph1{h}") for h in range(2)]
            for h in range(2):
                for k in range(2):
                    nc.tensor.matmul(out=ph1[h][:], lhsT=W1[:, k, h * 128:(h + 1) * 128],
                                     rhs=xT_t[:, k, :], start=(k == 0), stop=(k == 1))
            h1f = wp.tile([128, 2, 512], F32, tag="h1f")
            h1b = wp.tile([128, 2, 512], BF16, tag="h1b")
            for h in range(2):
                mn = wp.tile([128, 512], F32, tag="mn")
                nc.vector.tensor_scalar(out=mn[:], in0=ph1[h][:], scalar1=B1[:, h:h + 1],
                                        scalar2=0.0, op0=OP.add, op1=OP.min)
                ee = wp.tile([128, 512], F32, tag="ee")
                nc.scalar.activation(out=ee[:], in_=mn[:], func=AF.Exp)
                rr = wp.tile([128, 512], F32, tag="rr")
                nc.scalar.activation(out=rr[:], in_=ph1[h][:], func=AF.Relu, bias=B1[:, h:h + 1])
                nc.vector.tensor_tensor(out=h1f[:, h, :], in0=rr[:], in1=ee[:], op=OP.add)
                nc.gpsimd.tensor_copy(out=h1b[:, h, :], in_=h1f[:, h, :])
            for p4 in range(GPT):
                t = g * GPT + p4
                sl = slice(p4 * 128, (p4 + 1) * 128)
                pxd = pp.tile([128, 512], F32, tag="pxd")
                for k in range(2):
                    nc.tensor.matmul(out=pxd[:, 0:128], lhsT=h1b[:, k, sl], rhs=W2[:, k, :],
                                     start=(k == 0), stop=(k == 1))
                nc.tensor.matmul(out=pxd[:, 128:178], lhsT=ONES1_F[:], rhs=BRR[:],
                                 start=True, stop=False)
                for k in range(2):
                    nc.tensor.matmul(out=pxd[:, 128:178], lhsT=h1f[:, k, sl], rhs=WR[:, k, :],
                                     start=False, stop=(k == 1))
                rowt = wp.tile([128, 132], F32, tag="rowt")
                nc.vector.tensor_tensor(out=rowt[:, 0:128], in0=pxd[:, 0:128], in1=B2M[:],
                                        op=OP.add)
                # bf16-rounded copy of xd: na must be consistent with the bf16
                # values used for the pairwise matmul in E0
                xdb = wp.tile([128, 128], BF16, tag="xdb")
                nc.vector.tensor_copy(out=xdb[:], in_=rowt[:, 0:128])
                sq = wp.tile([128, 128], BF16, tag="sq")
                nc.scalar.activation(out=sq[:], in_=xdb[:], func=AF.Square,
                                     accum_out=rowt[:, 128:129])
                nc.vector.tensor_copy(out=rowt[:, 129:130], in_=msk_f[:, t:t + 1])
                nc.vector.memset(rowt[:, 130:132], 0.0)
                nc.vector.tensor_copy(out=mul_all[:, t, :], in_=pxd[:, 128:178])
                nc.sync.dma_start(out=xd_pm[t * 128:(t + 1) * 128, :], in_=rowt[:])
        ppA.__exit__(None, None, None)

        # ---------------- Phase B: argmax -> bins ----------------
        mx = wp.tile([128, NT], F32, tag="mx")
        nc.vector.tensor_reduce(out=mx[:], in_=mul_all[:], axis=mybir.AxisListType.X,
                                op=OP.max, apply_absolute_value=True)
        nmx = wp.tile([128, NT], F32, tag="nmx")
        nc.vector.tensor_scalar_mul(nmx[:], mx[:], -1.0)
        eqp = bigp.tile([128, NT, 50], F32, tag="O_all", name="eqp")
        eqn = mul_all
        nc.vector.tensor_tensor(out=eqp[:], in0=mul_all[:],
                                in1=mx[:].unsqueeze(2).to_broadcast([128, NT, 50]), op=OP.is_equal)
        nc.vector.tensor_tensor(out=eqn[:], in0=mul_all[:],
                                in1=nmx[:].unsqueeze(2).to_broadcast([128, NT, 50]), op=OP.is_equal)
        nc.vector.tensor_tensor(out=eqp[:], in0=eqp[:],
                                in1=ACST[:].unsqueeze(1).to_broadcast([128, NT, 50]), op=OP.mult)
        nc.vector.tensor_tensor(out=eqn[:], in0=eqn[:],
                                in1=BCST[:].unsqueeze(1).to_broadcast([128, NT, 50]), op=OP.mult)
        nc.vector.tensor_tensor(out=eqp[:], in0=eqp[:], in1=eqn[:], op=OP.max)
        rmx = wp.tile([128, NT], F32, tag="rmx")
        nc.vector.tensor_reduce(out=rmx[:], in_=eqp[:], axis=mybir.AxisListType.X, op=OP.max)
        m99 = wp.tile([128, NT], F32, tag="m99")
        nc.vector.tensor_scalar(out=m99[:], in0=msk_f[:], scalar1=-99.0, scalar2=99.0,
                                op0=OP.mult, op1=OP.add)
        binsf = wp.tile([128, NT], F32, tag="binsf")
        nc.vector.tensor_scalar(out=binsf[:], in0=rmx[:], scalar1=-1.0, scalar2=BIG,
                                op0=OP.mult, op1=OP.add)
        nc.vector.tensor_tensor(out=binsf[:], in0=binsf[:], in1=m99[:], op=OP.add)
        nc.vector.tensor_copy(out=bins_bf[:], in_=binsf[:])

        # ---------------- Phase C: ranks + inverse permutation ----------------
        for t in range(NT):
            nc.vector.tensor_tensor(out=O_all[:, t, :], in0=IOTAV[:],
                                    in1=bins_bf[:, t:t + 1].to_broadcast([128, V]),
                                    op=OP.is_equal)
        ppC = tc.tile_pool(name="psC", bufs=1, space="PSUM")
        pp1 = ppC.__enter__()
        ppC2 = tc.tile_pool(name="psC2", bufs=2, space="PSUM")
        pp = ppC2.__enter__()
        pG = pp1.tile([100, 512], F32, tag="pG")
        ph99 = pp1.tile([1, 512], F32, tag="ph99")
        for t in range(NT):
            nc.tensor.matmul(out=pG[:, 0:V], lhsT=UTREP[:, t, :], rhs=O_all[:, t, :],
                             start=(t == 0), stop=(t == NT - 1))
        for t in range(NT):
            nc.tensor.matmul(out=ph99[:, 0:V], lhsT=ONESC_BF[:], rhs=O_all[:, t, :],
                             start=(t == 0), stop=(t == NT - 1))
        Gpre = wp.tile([100, V], F32, tag="Gpre")
        nc.vector.tensor_copy(out=Gpre[:], in_=pG[:, 0:V])
        htot = wp.tile([1, V], F32, tag="htot")
        nc.vector.tensor_copy(out=htot[:], in_=ph99[:, 0:V])
        wz = wp.tile([1, V], F32, tag="wz")
        nc.vector.memset(wz[:], 0.0)
        incl = wp.tile([1, V], F32, tag="incl")
        nc.vector.tensor_tensor_scan(out=incl[:], data0=htot[:], data1=wz[:],
                                     initial=0.0, op0=OP.add, op1=OP.add)
        off = wp.tile([1, V], F32, tag="off")
        nc.vector.memset(off[:, 0:1], 0.0)
        nc.vector.tensor_copy(out=off[:, 1:V], in_=incl[:, 0:V - 1])
        pG2 = pp1.tile([100, 512], F32, tag="pG2")
        nc.tensor.matmul(out=pG2[:, 0:V], lhsT=ONES100_F[:], rhs=off[:], start=True, stop=False)
        nc.tensor.matmul(out=pG2[:, 0:V], lhsT=ID100[:], rhs=Gpre[:], start=False, stop=True)
        gi = wp.tile([100, V], I32, tag="gi")
        nc.vector.tensor_copy(out=gi[:], in_=pG2[:, 0:V])
        glo = wp.tile([100, V], I32, tag="glo")
        nc.vector.tensor_scalar(out=glo[:], in0=gi[:], scalar1=127, scalar2=None,
                                op0=OP.bitwise_and)
        ghi = wp.tile([100, V], I32, tag="ghi")
        nc.vector.tensor_scalar(out=ghi[:], in0=gi[:], scalar1=-128, scalar2=None,
                                op0=OP.bitwise_and)
        GG = wp.tile([100, 2 * V], BF16, tag="GG")
        nc.vector.tensor_copy(out=GG[:, 0:V], in_=glo[:])
        nc.vector.tensor_copy(out=GG[:, V:2 * V], in_=ghi[:])
        nc.sync.dma_start(out=gg_dr[:, :], in_=GG[:])

        # pass 2 (quartered GG to keep partition-0 footprint small); as each
        # tile's ranks land, scatter its iota to build the inverse permutation
        scr = wp.tile([128, 2 * V], BF16, tag="scr")
        for q in range(4):
            GG1 = bigp.tile([1, 25, 2 * V], BF16, tag="mul_all", name="GG1")
            nc.sync.dma_start(out=GG1[:], in_=gg_dr[q * 25:(q + 1) * 25, :].rearrange("t v -> (t v)").unsqueeze(0).rearrange("o (t v) -> o t v", t=25))
            for tq in range(25):
                t = q * 25 + tq
                pS = pp.tile([128, 512], F32, tag="pS")
                nc.tensor.matmul(out=pS[:, 0:2 * V], lhsT=ONES1_BF[:], rhs=GG1[:, tq, :],
                                 start=True, stop=False)
                nc.tensor.matmul(out=pS[:, 0:V], lhsT=UT128[:], rhs=O_all[:, t, :],
                                 start=False, stop=True)
                nc.vector.scalar_tensor_tensor(
                    out=scr[:], in0=pS[:, 0:2 * V].rearrange("p (c v) -> p c v", c=2), scalar=1.0,
                    in1=O_all[:, t, :].unsqueeze(1).to_broadcast([128, 2, V]),
                    op0=OP.mult, op1=OP.mult, accum_out=ranks_f[:, t:t + 1])
                nc.vector.tensor_copy(out=ranks_i[:, t:t + 1], in_=ranks_f[:, t:t + 1])
                nc.gpsimd.indirect_dma_start(
                    out=inv_dr[:, :],
                    out_offset=bass.IndirectOffsetOnAxis(ap=ranks_i[:, t:t + 1], axis=0),
                    in_=IOTA32[:, t:t + 1], in_offset=None)
        ppC2.__exit__(None, None, None)
        ppC.__exit__(None, None, None)

        # slot-major inverse permutation: inv_sm[p, c] = point at slot c*128+p
        nc.sync.dma_start(out=inv_sm[:], in_=inv_dr.rearrange("(c p) o -> p (c o)", p=128))

        # ---------------- Phase E0: per-bin gaussian adjacency ----------------
        CG = 4
        ppE = tc.tile_pool(name="psE0", bufs=2, space="PSUM")
        pp = ppE.__enter__()
        for g in range(NB // CG):
            idg = wp.tile([128, CG], F32, tag="idg")
            for j in range(CG):
                c = g * CG + j
                xds = wp.tile([128, 132], F32, tag=f"xds{j % 2}")
                nc.gpsimd.indirect_dma_start(
                    out=xds[:], out_offset=None, in_=xd_pm[:, :],
                    in_offset=bass.IndirectOffsetOnAxis(ap=inv_sm[:, c:c + 1], axis=0))
                pD = pp.tile([128, 384], F32, tag="pD")
                pNA = pp.tile([1, 128], F32, tag="pNA")
                nc.tensor.transpose(out=pD[:, 0:128], in_=xds[:, 0:128], identity=IDENT[:])
                nc.tensor.transpose(out=pNA[:], in_=xds[:, 128:129], identity=IDENT[:])
                xmT = wp.tile([128, 128], BF16, tag="xmT")
                nc.vector.tensor_copy(out=xmT[:], in_=pD[:, 0:128])
                xmT2 = wp.tile([128, 128], BF16, tag="xmT2")
                nc.vector.tensor_scalar_mul(xmT2[:], pD[:, 0:128], -2.0)
                naR = wp.tile([1, 128], F32, tag="naR")
                nc.vector.tensor_copy(out=naR[:], in_=pNA[:])
                nc.tensor.matmul(out=pD[:, 256:384], lhsT=xmT2[:], rhs=xmT[:],
                                 start=True, stop=False)
                nc.tensor.matmul(out=pD[:, 256:384], lhsT=ONES1_F[:], rhs=naR[:],
                                 start=False, stop=True)
                D2 = wp.tile([128, 128], F32, tag="D2")
                nc.vector.scalar_tensor_tensor(out=D2[:], in0=pD[:, 256:384],
                                               scalar=xds[:, 128:129],
                                               in1=E6[:].to_broadcast([128, 128]),
                                               op0=OP.add, op1=OP.max)
                nc.scalar.activation(out=D2[:], in_=D2[:], func=AF.Ln)
                nc.scalar.activation(out=D2[:], in_=D2[:], func=AF.Exp, scale=0.5)
                nc.scalar.activation(out=dm_all[:, c, :], in_=D2[:], func=AF.Exp, scale=-0.1)
                nc.vector.tensor_copy(out=msk_all[:, c:c + 1], in_=xds[:, 129:130])
                mskb = wp.tile([128, 1], BF16, tag="mskb")
                nc.vector.tensor_copy(out=mskb[:], in_=xds[:, 129:130])
                nc.tensor.matmul(out=pD[:, 132:133], lhsT=dm_all[:, c, :], rhs=mskb[:],
                                 start=True, stop=True)
                nc.vector.scalar_tensor_tensor(out=idg[:, j:j + 1], in0=pD[:, 132:133],
                                               scalar=xds[:, 129:130],
                                               in1=E6[:], op0=OP.mult, op1=OP.add)
            nc.scalar.activation(out=idg[:], in_=idg[:], func=AF.Ln)
            nc.scalar.activation(out=idg[:], in_=idg[:], func=AF.Exp, scale=-0.5)
            nc.vector.tensor_tensor(out=nco_all[:, g * CG:(g + 1) * CG], in0=idg[:],
                                    in1=msk_all[:, g * CG:(g + 1) * CG], op=OP.mult)
        ppE.__exit__(None, None, None)
        nc.vector.tensor_scalar_mul(mh_all[:], msk_all[:], 0.5)

        # ---------------- Phase E12: two fused GHConv layers per bin ----------------
        ppF = tc.tile_pool(name="psE12", bufs=2, space="PSUM")
        pp = ppF.__enter__()
        for c in range(NB):
            xbs = wp.tile([128, 256], BF16, tag="xbs")
            nc.gpsimd.indirect_dma_start(
                out=xbs[:], out_offset=None, in_=xb_d[:, :],
                in_offset=bass.IndirectOffsetOnAxis(ap=inv_sm[:, c:c + 1], axis=0))
            cur = xbs
            for l in range(2):
                pT = pp.tile([128, 256], BF16, tag="pT")
                xbT = wp.tile([128, 2, 128], BF16, tag="xbT")
                for k in range(2):
                    nc.tensor.transpose(out=pT[:, k * 128:(k + 1) * 128],
                                        in_=cur[:, k * 128:(k + 1) * 128], identity=IDENTB[:])
                nc.scalar.activation(out=xbT[:, 0, :], in_=pT[:, 0:128], func=AF.Copy)
                nc.vector.tensor_copy(out=xbT[:, 1, :], in_=pT[:, 128:256])
                pZH = pp.tile([128, 512], F32, tag="pZH")
                pPH = pp.tile([128, 512], F32, tag="pPH")
                nc.tensor.matmul(out=pZH[:, 0:512], lhsT=ONES1_BF[:], rhs=BTL[:, l, :],
                                 start=True, stop=False)
                for k in range(2):
                    nc.tensor.matmul(out=pZH[:, 0:512], lhsT=xbT[:, k, :],
                                     rhs=WCV[:, k, l, 0:512], start=False, stop=(k == 1))
                for k in range(2):
                    nc.tensor.matmul(out=pPH[:, 0:256], lhsT=xbT[:, k, :],
                                     rhs=WCV[:, k, l, 512:768], start=(k == 0), stop=(k == 1))
                tg = wp.tile([128, 256], BF16, tag="tg")
                if l == 0:
                    nc.scalar.activation(out=tg[:], in_=pZH[:, 0:256], func=AF.Tanh, scale=0.5)
                else:
                    nc.scalar.activation(out=tg[:], in_=pZH[:, 0:256], func=AF.Tanh,
                                         scale=mh_all[:, c:c + 1])
                h2 = wp.tile([128, 256], BF16, tag="h2")
                nc.scalar.activation(out=h2[:], in_=pPH[:, 0:256], func=AF.Copy,
                                     scale=nco_all[:, c:c + 1])
                nc.tensor.matmul(out=pPH[:, 256:512], lhsT=dm_all[:, c, :], rhs=h2[:],
                                 start=True, stop=True)
                eh = wp.tile([128, 256], BF16, tag="eh")
                nc.scalar.activation(out=eh[:], in_=pZH[:, 256:512], func=AF.Copy)
                dd = wp.tile([128, 256], BF16, tag="dd")
                nc.vector.scalar_tensor_tensor(out=dd[:], in0=pPH[:, 256:512],
                                               scalar=nco_all[:, c:c + 1],
                                               in1=eh[:], op0=OP.mult,
                                               op1=OP.subtract)
                w1u = wp.tile([128, 256], BF16, tag="w1u")
                nc.vector.scalar_tensor_tensor(out=w1u[:], in0=tg[:], scalar=1.0,
                                               in1=dd[:], op0=OP.add, op1=OP.mult)
                ob = wp.tile([128, 256], BF16, tag="ob")
                nc.vector.scalar_tensor_tensor(out=ob[:], in0=w1u[:], scalar=0.5,
                                               in1=eh[:], op0=OP.mult, op1=OP.add)
                ex2 = wp.tile([128, 256], BF16, tag="ex2")
                nc.scalar.activation(out=ex2[:], in_=ob[:], func=AF.Exp)
                exm = wp.tile([128, 256], BF16, tag="exm")
                nc.vector.tensor_scalar(out=exm[:], in0=ex2[:], scalar1=1.0, scalar2=-1.0,
                                        op0=OP.min, op1=OP.add)
                if l == 0:
                    nxt = wp.tile([128, 256], BF16, tag="nxt")
                    nc.vector.scalar_tensor_tensor(out=nxt[:], in0=ob[:], scalar=0.0,
                                                   in1=exm[:], op0=OP.max, op1=OP.add)
                    cur = nxt
                else:
                    fin0 = wp.tile([128, 256], BF16, tag="fin0")
                    nc.vector.scalar_tensor_tensor(out=fin0[:], in0=ob[:], scalar=0.0,
                                                   in1=exm[:], op0=OP.max, op1=OP.add)
                    fin = wp.tile([128, 256], F32, tag="fin")
                    nc.vector.tensor_scalar(out=fin[:], in0=fin0[:],
                                            scalar1=msk_all[:, c:c + 1], scalar2=None,
                                            op0=OP.mult)
                    nc.gpsimd.indirect_dma_start(
                        out=out_d[:, :],
                        out_offset=bass.IndirectOffsetOnAxis(ap=inv_sm[:, c:c + 1], axis=0),
                        in_=fin[:], in_offset=None)
        ppF.__exit__(None, None, None)


def build_nc(consts):
    nc = bacc.Bacc(None, target_bir_lowering=False)
    xT_d = [nc.dram_tensor(f"xT{b}", [F, N], F32, kind="ExternalInput") for b in range(BPC)]
    xb_d = [nc.dram_tensor(f"xb{b}", [N, F], BF16, kind="ExternalInput") for b in range(BPC)]
    msk_d = [nc.dram_tensor(f"msk{b}", [N], U8, kind="ExternalInput") for b in range(BPC)]
    out_d = [nc.dram_tensor(f"out{b}", [N, F], F32, kind="ExternalOutput") for b in range(BPC)]
    xd_pm = [nc.dram_tensor(f"xdpm{b}", [N, 132], F32) for b in range(BPC)]
    inv_dr = [nc.dram_tensor(f"invdr{b}", [N, 1], I32) for b in range(BPC)]
    gg_dr = [nc.dram_tensor(f"ggdr{b}", [100, 2 * V], BF16) for b in range(BPC)]

    cd = {k: nc.inline_tensor(v, name=k) for k, v in consts.items()}
    with TileContext(nc) as tc:
        with tc.tile_pool(name="const", bufs=1) as cp:
            ct = {}
            for k, v in consts.items():
                t = cp.tile(list(v.shape), mybir.dt.from_np(v.dtype), tag=k, name=k)
                nc.sync.dma_start(out=t[:], in_=cd[k][:])
                ct[k] = t
            for b in range(BPC):
                build_batch(nc, tc, ct, xT_d[b], xb_d[b], msk_d[b], out_d[b],
                            xd_pm[b], inv_dr[b], gg_dr[b])
    nc.finalize()
    return nc


def make_consts(w_d1, b_d1, w_d2, b_d2, rot, wt0, bt0, wh0, th0, wt1, bt1, wh1, th1):
    rot50 = rot[:, :NB // 2]
    WR = (w_d2 @ rot50).astype(np.float32)
    # phase A produces h1' = elu+1; fold the -1 into downstream biases
    b2_eff = (b_d2 - w_d2.sum(0)).astype(np.float32)
    BRR = (b2_eff @ rot50).astype(np.float32)[None]

    jj = np.arange(50, dtype=np.float32)
    ACST = (BIG - jj)[None].repeat(128, 0).astype(np.float32)
    BCST = (BIG - 50.0 - jj)[None].repeat(128, 0).astype(np.float32)
    IOTAV = np.arange(V, dtype=np.float32)[None].repeat(128, 0).astype(bf)
    UT128 = (np.arange(128)[:, None] < np.arange(128)[None, :]).astype(bf)
    ut = (np.arange(NB)[:, None] < np.arange(NB)[None, :]).astype(bf)  # [t, tt]=1 iff t<tt
    UTREP = np.broadcast_to(ut[None], (128, NB, NB)).copy()

    W1 = w_d1.reshape(2, 128, 256).transpose(1, 0, 2).astype(np.float32).copy()
    B1 = b_d1.reshape(2, 128).T.astype(np.float32).copy()
    W2 = w_d2.reshape(2, 128, 128).transpose(1, 0, 2).astype(bf).copy()
    B2M = b2_eff[None].repeat(128, 0).astype(np.float32)
    WRc = WR.reshape(2, 128, 50).transpose(1, 0, 2).astype(np.float32).copy()

    # WCV[:, k, l, 0:256]=wt_l, 256:512=wh_l, 512:768=th_l  (k-split rows)
    wcv = np.zeros((128, 2, 2, 768), dtype=bf)
    for l, (wt, wh, th) in enumerate([(wt0, wh0, th0), (wt1, wh1, th1)]):
        for i, w in enumerate([wt, wh, th]):
            wc = w.reshape(2, 128, 256)
            wcv[:, 0, l, 256 * i:256 * (i + 1)] = wc[0].astype(bf)
            wcv[:, 1, l, 256 * i:256 * (i + 1)] = wc[1].astype(bf)
    # bias rows: [bt_l | zeros]  (wt path gets bias, wh path none)
    btl = np.zeros((1, 2, 512), dtype=bf)
    btl[0, 0, 0:256] = bt0.astype(bf)
    btl[0, 1, 0:256] = bt1.astype(bf)

    iota32 = (np.arange(NT, dtype=np.int32)[None, :] * 128
              + np.arange(128, dtype=np.int32)[:, None]).astype(np.int32)

    return {
        "W1": W1, "B1": B1, "W2": W2, "B2M": B2M, "WR": WRc, "BRR": BRR,
        "ACST": ACST, "BCST": BCST, "IOTAV": IOTAV, "UT128": UT128, "UTREP": UTREP,
        "ONES1_BF": np.ones((1, 128), dtype=bf),
        "ONES1_F": np.ones((1, 128), dtype=np.float32),
        "ONESC_BF": np.ones((128, 1), dtype=bf),
        "ONES100_F": np.ones((1, 100), dtype=np.float32),
        "IDENT": np.eye(128, dtype=np.float32),
        "IDENTB": np.eye(128, dtype=bf),
        "ID100": np.eye(100, dtype=np.float32),
        "E6": np.full((128, 1), 1e-6, dtype=np.float32),
        "WCV": wcv, "BTL": btl, "IOTA32": iota32,
    }


_cache = {}


def kernel(x, msk, rot, w_d1, b_d1, w_d2, b_d2, wt0, bt0, wh0, th0, wt1, bt1, wh1, th1):
    x = np.asarray(x, np.float32)
    msk_u8 = np.asarray(msk).astype(np.uint8)
    consts = make_consts(np.asarray(w_d1, np.float32), np.asarray(b_d1, np.float32),
                         np.asarray(w_d2, np.float32), np.asarray(b_d2, np.float32),
                         np.asarray(rot, np.float32),
                         np.asarray(wt0, np.float32), np.asarray(bt0, np.float32),
                         np.asarray(wh0, np.float32), np.asarray(th0, np.float32),
                         np.asarray(wt1, np.float32), np.asarray(bt1, np.float32),
                         np.asarray(wh1, np.float32), np.asarray(th1, np.float32))
    if "nc" not in _cache:
        _cache["nc"] = build_nc(consts)
    nc = _cache["nc"]

    xT = np.ascontiguousarray(x.transpose(0, 2, 1))
    xb16 = np.ascontiguousarray(x.astype(bf))

    in_maps = []
    for c in range(NCORES):
        m = {}
        for b in range(BPC):
            gb = c * BPC + b
            m[f"xT{b}"] = xT[gb]
            m[f"xb{b}"] = xb16[gb]
            m[f"msk{b}"] = msk_u8[gb]
        in_maps.append(m)

    import os as _os
    _trace = _os.environ.get("KERNEL_TRACE", "") == "1"
    res = run_bass_kernel_spmd(nc, in_maps, core_ids=list(range(NCORES)), trace=_trace)
    if _trace:
        print(f"HW exec time: {res.exec_time_ns} ns")
        _cache["exec_time_ns"] = res.exec_time_ns
        _cache["res"] = res
    out = np.empty((B, N, F), np.float32)
    for c in range(NCORES):
        for b in range(BPC):
            out[c * BPC + b] = res.results[c][f"out{b}"]
    return out


# revision 16
# speedup vs baseline: 1.3686x; 1.1909x over previous
"""Trainium2 Bass kernel for nn_CombinedGraphLayer (LSH-binned GNN message passing).

Full inputs in, full output out. Shards batch (B=16) over 8 NeuronCores (2 per core).
Per batch on device:
  A) ffn_dist (fp32 on the LSH-critical path), writes xd rows (xd|na|msk) to DRAM,
     keeps rot `mul` in SBUF
  B) argmax -> bin index per point
  C) counting-sort ranks (exact integer arithmetic via one-hot + triangular matmuls);
     iota-scatter by rank builds the inverse permutation inv (slot -> point)
  E0) per-bin: indirect-gather xd rows by inv, pairwise gaussian adjacency (bf16
      matmul, consistent na), degree + norm.  Only Ln/Exp activations.
  E12) per-bin: indirect-gather x rows by inv, two fused GHConv layers (bf16),
      tanh gate + exp elu (same act table set), final rows indirect-scattered
      straight to the output by inv.  No slot-major DRAM round trips.
"""
import sys
sys.path.insert(0, "/opt/trn_rl_repo")
import numpy as np
import ml_dtypes

import concourse.bass as bass
import concourse.bacc as bacc
import concourse.mybir as mybir
from concourse.tile import TileContext
from concourse.bass_utils import run_bass_kernel_spmd

F32 = mybir.dt.float32
BF16 = mybir.dt.bfloat16
I32 = mybir.dt.int32
U8 = mybir.dt.uint8
AF = mybir.ActivationFunctionType
OP = mybir.AluOpType

B, N, F = 16, 12800, 256
BIN = 128
NB = N // BIN          # 100 bins
NCORES = 8
BPC = B // NCORES      # 2 batches per core
NT = N // 128          # 100 point tiles
BIG = 1000.0
V = 200                # one-hot width (bin ids 0..198)

bf = ml_dtypes.bfloat16


def build_batch(nc, tc, ct, xT_d, xb_d, msk_d, out_d, xd_pm, gg_dr):
    W1, B1, W2, B2M, WR, BRR = ct["W1"], ct["B1"], ct["W2"], ct["B2M"], ct["WR"], ct["BRR"]
    ACST, BCST, IOTAV = ct["ACST"], ct["BCST"], ct["IOTAV"]
    UT128, UTREP = ct["UT128"], ct["UTREP"]
    ONES1_BF, ONES1_F, ONESC_BF, ONES100_F = ct["ONES1_BF"], ct["ONES1_F"], ct["ONESC_BF"], ct["ONES100_F"]
    IDENT, IDENTB, ID100, E6 = ct["IDENT"], ct["IDENTB"], ct["ID100"], ct["E6"]
    WCV, BTL = ct["WCV"], ct["BTL"]

    with tc.tile_pool(name="big", bufs=1) as bigp, \
         tc.tile_pool(name="wrk", bufs=2) as wp:

        mul_all = bigp.tile([128, NT, 50], F32, tag="mul_all")
        bins_bf = bigp.tile([128, NT], BF16, tag="bins_bf")
        ranks_f = bigp.tile([128, NT], F32, tag="ranks_f")
        ranks_i = bigp.tile([128, NT], I32, tag="ranks_i")
        O_all = bigp.tile([128, NT, V], BF16, tag="O_all")
        msk_f = bigp.tile([128, NT], F32, tag="msk_f")
        dm_all = bigp.tile([128, NB, 128], BF16, tag="dm_all")
        nco_all = bigp.tile([128, NB], F32, tag="nco_all")
        msk_all = bigp.tile([128, NB], F32, tag="msk_all")
        mh_all = bigp.tile([128, NB], F32, tag="mh_all")
        inv_sm = bigp.tile([128, NB], I32, tag="inv_sm")

        msk_u8 = wp.tile([128, NT], U8)
        nc.sync.dma_start(out=msk_u8[:], in_=msk_d.rearrange("(t p) -> p t", p=128))
        nc.vector.tensor_copy(out=msk_f[:], in_=msk_u8[:])

        # ---------------- Phase A: ffn + rot ----------------
        GPT = 4
        ppA = tc.tile_pool(name="psA", bufs=2, space="PSUM")
        pp = ppA.__enter__()
        for g in range(NT // GPT):
            xT_t = wp.tile([128, 2, GPT * 128], F32, tag="xT")
            nc.sync.dma_start(
                out=xT_t[:],
                in_=xT_d[:, g * GPT * 128:(g + 1) * GPT * 128].rearrange("(c p) n -> p c n", p=128))
            ph1 = [pp.tile([128, 512], F32, tag=f"ph1{h}", name=f"ph1{h}") for h in range(2)]
            for h in range(2):
                for k in range(2):
                    nc.tensor.matmul(out=ph1[h][:], lhsT=W1[:, k, h * 128:(h + 1) * 128],
                                     rhs=xT_t[:, k, :], start=(k == 0), stop=(k == 1))
            h1f = wp.tile([128, 2, 512], F32, tag="h1f")
            h1b = wp.tile([128, 2, 512], BF16, tag="h1b")
            for h in range(2):
                mn = wp.tile([128, 512], F32, tag="mn")
                nc.vector.tensor_scalar(out=mn[:], in0=ph1[h][:], scalar1=B1[:, h:h + 1],
                                        scalar2=0.0, op0=OP.add, op1=OP.min)
                ee = wp.tile([128, 512], F32, tag="ee")
                nc.scalar.activation(out=ee[:], in_=mn[:], func=AF.Exp)
                rr = wp.tile([128, 512], F32, tag="rr")
                nc.scalar.activation(out=rr[:], in_=ph1[h][:], func=AF.Relu, bias=B1[:, h:h + 1])
                nc.vector.tensor_tensor(out=h1f[:, h, :], in0=rr[:], in1=ee[:], op=OP.add)
                nc.gpsimd.tensor_copy(out=h1b[:, h, :], in_=h1f[:, h, :])
            for p4 in range(GPT):
                t = g * GPT + p4
                sl = slice(p4 * 128, (p4 + 1) * 128)
                pxd = pp.tile([128, 512], F32, tag="pxd")
                for k in range(2):
                    nc.tensor.matmul(out=pxd[:, 0:128], lhsT=h1b[:, k, sl], rhs=W2[:, k, :],
                                     start=(k == 0), stop=(k == 1))
                nc.tensor.matmul(out=pxd[:, 128:178], lhsT=ONES1_F[:], rhs=BRR[:],
                                 start=True, stop=False)
                for k in range(2):
                    nc.tensor.matmul(out=pxd[:, 128:178], lhsT=h1f[:, k, sl], rhs=WR[:, k, :],
                                     start=False, stop=(k == 1))
                rowt = wp.tile([128, 132], F32, tag="rowt")
                nc.vector.tensor_tensor(out=rowt[:, 0:128], in0=pxd[:, 0:128], in1=B2M[:],
                                        op=OP.add)
                # bf16-rounded copy of xd: na must be consistent with the bf16
                # values used for the pairwise matmul in E0
                xdb = wp.tile([128, 128], BF16, tag="xdb")
                nc.vector.tensor_copy(out=xdb[:], in_=rowt[:, 0:128])
                sq = wp.tile([128, 128], BF16, tag="sq")
                nc.scalar.activation(out=sq[:], in_=xdb[:], func=AF.Square,
                                     accum_out=rowt[:, 128:129])
                nc.vector.tensor_copy(out=rowt[:, 129:130], in_=msk_f[:, t:t + 1])
                nc.vector.memset(rowt[:, 130:132], 0.0)
                nc.vector.tensor_copy(out=mul_all[:, t, :], in_=pxd[:, 128:178])
                nc.sync.dma_start(out=xd_pm[t * 128:(t + 1) * 128, :], in_=rowt[:])
        ppA.__exit__(None, None, None)

        # ---------------- Phase B: argmax -> bins ----------------
        mx = wp.tile([128, NT], F32, tag="mx")
        nc.vector.tensor_reduce(out=mx[:], in_=mul_all[:], axis=mybir.AxisListType.X,
                                op=OP.max, apply_absolute_value=True)
        nmx = wp.tile([128, NT], F32, tag="nmx")
        nc.vector.tensor_scalar_mul(nmx[:], mx[:], -1.0)
        eqp = bigp.tile([128, NT, 50], F32, tag="O_all", name="eqp")
        eqn = mul_all
        nc.vector.tensor_tensor(out=eqp[:], in0=mul_all[:],
                                in1=mx[:].unsqueeze(2).to_broadcast([128, NT, 50]), op=OP.is_equal)
        nc.vector.tensor_tensor(out=eqn[:], in0=mul_all[:],
                                in1=nmx[:].unsqueeze(2).to_broadcast([128, NT, 50]), op=OP.is_equal)
        nc.vector.tensor_tensor(out=eqp[:], in0=eqp[:],
                                in1=ACST[:].unsqueeze(1).to_broadcast([128, NT, 50]), op=OP.mult)
        nc.vector.tensor_tensor(out=eqn[:], in0=eqn[:],
                                in1=BCST[:].unsqueeze(1).to_broadcast([128, NT, 50]), op=OP.mult)
        nc.vector.tensor_tensor(out=eqp[:], in0=eqp[:], in1=eqn[:], op=OP.max)
        rmx = wp.tile([128, NT], F32, tag="rmx")
        nc.vector.tensor_reduce(out=rmx[:], in_=eqp[:], axis=mybir.AxisListType.X, op=OP.max)
        m99 = wp.tile([128, NT], F32, tag="m99")
        nc.vector.tensor_scalar(out=m99[:], in0=msk_f[:], scalar1=-99.0, scalar2=99.0,
                                op0=OP.mult, op1=OP.add)
        binsf = wp.tile([128, NT], F32, tag="binsf")
        nc.vector.tensor_scalar(out=binsf[:], in0=rmx[:], scalar1=-1.0, scalar2=BIG,
                                op0=OP.mult, op1=OP.add)
        nc.vector.tensor_tensor(out=binsf[:], in0=binsf[:], in1=m99[:], op=OP.add)
        nc.vector.tensor_copy(out=bins_bf[:], in_=binsf[:])

        # ---------------- Phase C: ranks + inverse permutation ----------------
        for t in range(NT):
            nc.vector.tensor_tensor(out=O_all[:, t, :], in0=IOTAV[:],
                                    in1=bins_bf[:, t:t + 1].to_broadcast([128, V]),
                                    op=OP.is_equal)
        ppC = tc.tile_pool(name="psC", bufs=1, space="PSUM")
        pp1 = ppC.__enter__()
        ppC2 = tc.tile_pool(name="psC2", bufs=2, space="PSUM")
        pp = ppC2.__enter__()
        pG = pp1.tile([100, 512], F32, tag="pG")
        ph99 = pp1.tile([1, 512], F32, tag="ph99")
        for t in range(NT):
            nc.tensor.matmul(out=pG[:, 0:V], lhsT=UTREP[:, t, :], rhs=O_all[:, t, :],
                             start=(t == 0), stop=(t == NT - 1))
        for t in range(NT):
            nc.tensor.matmul(out=ph99[:, 0:V], lhsT=ONESC_BF[:], rhs=O_all[:, t, :],
                             start=(t == 0), stop=(t == NT - 1))
        Gpre = wp.tile([100, V], F32, tag="Gpre")
        nc.vector.tensor_copy(out=Gpre[:], in_=pG[:, 0:V])
        htot = wp.tile([1, V], F32, tag="htot")
        nc.vector.tensor_copy(out=htot[:], in_=ph99[:, 0:V])
        wz = wp.tile([1, V], F32, tag="wz")
        nc.vector.memset(wz[:], 0.0)
        incl = wp.tile([1, V], F32, tag="incl")
        nc.vector.tensor_tensor_scan(out=incl[:], data0=htot[:], data1=wz[:],
                                     initial=0.0, op0=OP.add, op1=OP.add)
        off = wp.tile([1, V], F32, tag="off")
        nc.vector.memset(off[:, 0:1], 0.0)
        nc.vector.tensor_copy(out=off[:, 1:V], in_=incl[:, 0:V - 1])
        pG2 = pp1.tile([100, 512], F32, tag="pG2")
        nc.tensor.matmul(out=pG2[:, 0:V], lhsT=ONES100_F[:], rhs=off[:], start=True, stop=False)
        nc.tensor.matmul(out=pG2[:, 0:V], lhsT=ID100[:], rhs=Gpre[:], start=False, stop=True)
        gi = wp.tile([100, V], I32, tag="gi")
        nc.vector.tensor_copy(out=gi[:], in_=pG2[:, 0:V])
        glo = wp.tile([100, V], I32, tag="glo")
        nc.vector.tensor_scalar(out=glo[:], in0=gi[:], scalar1=127, scalar2=None,
                                op0=OP.bitwise_and)
        ghi = wp.tile([100, V], I32, tag="ghi")
        nc.vector.tensor_scalar(out=ghi[:], in0=gi[:], scalar1=-128, scalar2=None,
                                op0=OP.bitwise_and)
        GG = wp.tile([100, 2 * V], BF16, tag="GG")
        nc.vector.tensor_copy(out=GG[:, 0:V], in_=glo[:])
        nc.vector.tensor_copy(out=GG[:, V:2 * V], in_=ghi[:])
        nc.sync.dma_start(out=gg_dr[:, :], in_=GG[:])

        # pass 2 (quartered GG to keep partition-0 footprint small); as each
        # tile's ranks land, accumulate the inverse permutation via matmul:
        # inv_sm[j, c] = sum_p pid[p,t] * [rank%128 == j] * [rank//128 == c]
        IOTAROW_I, IOTA100_I, PIDF = ct["IOTAROW_I"], ct["IOTA100_I"], ct["PIDF"]
        pINV = pp1.tile([128, 100], F32, tag="pINV")
        scr = wp.tile([128, 2 * V], BF16, tag="scr")
        for q in range(4):
            GG1 = bigp.tile([1, 25, 2 * V], BF16, tag="mul_all", name="GG1")
            nc.sync.dma_start(out=GG1[:], in_=gg_dr[q * 25:(q + 1) * 25, :].rearrange("t v -> (t v)").unsqueeze(0).rearrange("o (t v) -> o t v", t=25))
            for tq in range(25):
                t = q * 25 + tq
                pS = pp.tile([128, 512], F32, tag="pS")
                nc.tensor.matmul(out=pS[:, 0:2 * V], lhsT=ONES1_BF[:], rhs=GG1[:, tq, :],
                                 start=True, stop=False)
                nc.tensor.matmul(out=pS[:, 0:V], lhsT=UT128[:], rhs=O_all[:, t, :],
                                 start=False, stop=True)
                nc.vector.scalar_tensor_tensor(
                    out=scr[:], in0=pS[:, 0:2 * V].rearrange("p (c v) -> p c v", c=2), scalar=1.0,
                    in1=O_all[:, t, :].unsqueeze(1).to_broadcast([128, 2, V]),
                    op0=OP.mult, op1=OP.mult, accum_out=ranks_f[:, t:t + 1])
                nc.vector.tensor_copy(out=ranks_i[:, t:t + 1], in_=ranks_f[:, t:t + 1])
                rmod = wp.tile([128, 1], I32, tag="rmod")
                nc.vector.tensor_scalar(out=rmod[:], in0=ranks_i[:, t:t + 1], scalar1=127,
                                        scalar2=None, op0=OP.bitwise_and)
                rdiv = wp.tile([128, 1], I32, tag="rdiv")
                nc.vector.tensor_scalar(out=rdiv[:], in0=ranks_i[:, t:t + 1], scalar1=7,
                                        scalar2=None, op0=OP.logical_shift_right)
                Apid = wp.tile([128, 128], F32, tag="Apid")
                nc.vector.scalar_tensor_tensor(
                    out=Apid[:], in0=IOTAROW_I[:], scalar=rmod[:],
                    in1=PIDF[:, t:t + 1].to_broadcast([128, 128]),
                    op0=OP.is_equal, op1=OP.mult)
                Bsel = wp.tile([128, 100], F32, tag="Bsel")
                nc.vector.tensor_tensor(out=Bsel[:], in0=IOTA100_I[:],
                                        in1=rdiv[:].to_broadcast([128, 100]),
                                        op=OP.is_equal)
                nc.tensor.matmul(out=pINV[:], lhsT=Apid[:], rhs=Bsel[:],
                                 start=(t == 0), stop=(t == NT - 1))
        nc.vector.tensor_copy(out=inv_sm[:], in_=pINV[:])
        ppC2.__exit__(None, None, None)
        ppC.__exit__(None, None, None)

        # ---------------- Phase E0a: pairwise distances (Sqrt only) ----------------
        mskb_all = bigp.tile([128, NB], BF16, tag="mskb_all")
        idg_all = bigp.tile([128, NB], F32, tag="idg_all")
        ppE = tc.tile_pool(name="psE0", bufs=3, space="PSUM")
        pp = ppE.__enter__()
        for c in range(NB):
            xds = wp.tile([128, 132], F32, tag=f"xds{c % 3}")
            nc.gpsimd.indirect_dma_start(
                out=xds[:], out_offset=None, in_=xd_pm[:, :],
                in_offset=bass.IndirectOffsetOnAxis(ap=inv_sm[:, c:c + 1], axis=0))
            pD = pp.tile([128, 384], F32, tag="pD")
            pNA = pp.tile([1, 128], F32, tag="pNA")
            nc.tensor.transpose(out=pD[:, 0:128], in_=xds[:, 0:128], identity=IDENT[:])
            nc.tensor.transpose(out=pNA[:], in_=xds[:, 128:129], identity=IDENT[:])
            xmT = wp.tile([128, 128], BF16, tag="xmT")
            nc.vector.tensor_copy(out=xmT[:], in_=pD[:, 0:128])
            xmT2 = wp.tile([128, 128], BF16, tag="xmT2")
            nc.vector.tensor_scalar_mul(xmT2[:], pD[:, 0:128], -2.0)
            naR = wp.tile([1, 128], F32, tag="naR")
            nc.vector.tensor_copy(out=naR[:], in_=pNA[:])
            nc.tensor.matmul(out=pD[:, 256:384], lhsT=xmT2[:], rhs=xmT[:],
                             start=True, stop=False)
            nc.tensor.matmul(out=pD[:, 256:384], lhsT=ONES1_F[:], rhs=naR[:],
                             start=False, stop=True)
            D2 = wp.tile([128, 128], F32, tag="D2")
            nc.vector.scalar_tensor_tensor(out=D2[:], in0=pD[:, 256:384],
                                           scalar=xds[:, 128:129],
                                           in1=E6[:].to_broadcast([128, 128]),
                                           op0=OP.add, op1=OP.max)
            nc.scalar.activation(out=dm_all[:, c, :], in_=D2[:], func=AF.Sqrt)
            nc.vector.tensor_copy(out=msk_all[:, c:c + 1], in_=xds[:, 129:130])
            nc.vector.tensor_copy(out=mskb_all[:, c:c + 1], in_=xds[:, 129:130])
        ppE.__exit__(None, None, None)

        # ---------------- Phase E0b: dm = exp(-0.1 d), degree ----------------
        ppE2 = tc.tile_pool(name="psE0b", bufs=3, space="PSUM")
        pp = ppE2.__enter__()
        for c in range(NB):
            nc.scalar.activation(out=dm_all[:, c, :], in_=dm_all[:, c, :], func=AF.Exp,
                                 scale=-0.1)
            pDG = pp.tile([128, 2], F32, tag="pDG")
            nc.tensor.matmul(out=pDG[:, 0:1], lhsT=dm_all[:, c, :],
                             rhs=mskb_all[:, c:c + 1], start=True, stop=True)
            nc.vector.scalar_tensor_tensor(out=idg_all[:, c:c + 1], in0=pDG[:, 0:1],
                                           scalar=msk_all[:, c:c + 1],
                                           in1=E6[:], op0=OP.mult, op1=OP.add)
        ppE2.__exit__(None, None, None)
        nc.vector.reciprocal(out=idg_all[:], in_=idg_all[:])
        nc.scalar.activation(out=idg_all[:], in_=idg_all[:], func=AF.Sqrt)
        nc.vector.tensor_tensor(out=nco_all[:], in0=idg_all[:], in1=msk_all[:], op=OP.mult)
        nc.vector.tensor_scalar_mul(mh_all[:], msk_all[:], 0.5)

        # ---------------- Phase E12: two fused GHConv layers per bin ----------------
        ppF = tc.tile_pool(name="psE12", bufs=2, space="PSUM")
        pp = ppF.__enter__()
        for c in range(NB):
            xbs = wp.tile([128, 256], BF16, tag=f"xbs{c % 3}")
            nc.gpsimd.indirect_dma_start(
                out=xbs[:], out_offset=None, in_=xb_d[:, :],
                in_offset=bass.IndirectOffsetOnAxis(ap=inv_sm[:, c:c + 1], axis=0))
            cur = xbs
            for l in range(2):
                pT = pp.tile([128, 256], BF16, tag="pT")
                xbT = wp.tile([128, 2, 128], BF16, tag="xbT")
                for k in range(2):
                    nc.tensor.transpose(out=pT[:, k * 128:(k + 1) * 128],
                                        in_=cur[:, k * 128:(k + 1) * 128], identity=IDENTB[:])
                nc.scalar.activation(out=xbT[:, 0, :], in_=pT[:, 0:128], func=AF.Copy)
                nc.vector.tensor_copy(out=xbT[:, 1, :], in_=pT[:, 128:256])
                pZH = pp.tile([128, 512], F32, tag="pZH")
                pPH = pp.tile([128, 512], F32, tag="pPH")
                nc.tensor.matmul(out=pZH[:, 0:512], lhsT=ONES1_BF[:], rhs=BTL[:, l, :],
                                 start=True, stop=False)
                for k in range(2):
                    nc.tensor.matmul(out=pZH[:, 0:512], lhsT=xbT[:, k, :],
                                     rhs=WCV[:, k, l, 0:512], start=False, stop=(k == 1))
                for k in range(2):
                    nc.tensor.matmul(out=pPH[:, 0:256], lhsT=xbT[:, k, :],
                                     rhs=WCV[:, k, l, 512:768], start=(k == 0), stop=(k == 1))
                tg = wp.tile([128, 256], BF16, tag="tg")
                if l == 0:
                    nc.scalar.activation(out=tg[:], in_=pZH[:, 0:256], func=AF.Tanh, scale=0.5)
                else:
                    nc.scalar.activation(out=tg[:], in_=pZH[:, 0:256], func=AF.Tanh,
                                         scale=mh_all[:, c:c + 1])
                h2 = wp.tile([128, 256], BF16, tag="h2")
                nc.scalar.activation(out=h2[:], in_=pPH[:, 0:256], func=AF.Copy,
                                     scale=nco_all[:, c:c + 1])
                nc.tensor.matmul(out=pPH[:, 256:512], lhsT=dm_all[:, c, :], rhs=h2[:],
                                 start=True, stop=True)
                eh = wp.tile([128, 256], BF16, tag="eh")
                nc.scalar.activation(out=eh[:], in_=pZH[:, 256:512], func=AF.Copy)
                dd = wp.tile([128, 256], BF16, tag="dd")
                nc.vector.scalar_tensor_tensor(out=dd[:], in0=pPH[:, 256:512],
                                               scalar=nco_all[:, c:c + 1],
                                               in1=eh[:], op0=OP.mult,
                                               op1=OP.subtract)
                w1u = wp.tile([128, 256], BF16, tag="w1u")
                nc.vector.scalar_tensor_tensor(out=w1u[:], in0=tg[:], scalar=1.0,
                                               in1=dd[:], op0=OP.add, op1=OP.mult)
                ob = wp.tile([128, 256], BF16, tag="ob")
                nc.vector.scalar_tensor_tensor(out=ob[:], in0=w1u[:], scalar=0.5,
                                               in1=eh[:], op0=OP.mult, op1=OP.add)
                ex2 = wp.tile([128, 256], BF16, tag="ex2")
                nc.scalar.activation(out=ex2[:], in_=ob[:], func=AF.Exp)
                exm = wp.tile([128, 256], BF16, tag="exm")
                nc.vector.tensor_scalar(out=exm[:], in0=ex2[:], scalar1=1.0, scalar2=-1.0,
                                        op0=OP.min, op1=OP.add)
                if l == 0:
                    nxt = wp.tile([128, 256], BF16, tag="nxt")
                    nc.vector.scalar_tensor_tensor(out=nxt[:], in0=ob[:], scalar=0.0,
                                                   in1=exm[:], op0=OP.max, op1=OP.add)
                    cur = nxt
                else:
                    fin0 = wp.tile([128, 256], BF16, tag="fin0")
                    nc.vector.scalar_tensor_tensor(out=fin0[:], in0=ob[:], scalar=0.0,
                                                   in1=exm[:], op0=OP.max, op1=OP.add)
                    fin = wp.tile([128, 256], F32, tag="fin")
                    nc.vector.tensor_scalar(out=fin[:], in0=fin0[:],
                                            scalar1=msk_all[:, c:c + 1], scalar2=None,
                                            op0=OP.mult)
                    nc.gpsimd.indirect_dma_start(
                        out=out_d[:, :],
                        out_offset=bass.IndirectOffsetOnAxis(ap=inv_sm[:, c:c + 1], axis=0),
                        in_=fin[:], in_offset=None)
        ppF.__exit__(None, None, None)


def build_nc(consts):
    nc = bacc.Bacc(None, target_bir_lowering=False)
    xT_d = [nc.dram_tensor(f"xT{b}", [F, N], F32, kind="ExternalInput") for b in range(BPC)]
    xb_d = [nc.dram_tensor(f"xb{b}", [N, F], BF16, kind="ExternalInput") for b in range(BPC)]
    msk_d = [nc.dram_tensor(f"msk{b}", [N], U8, kind="ExternalInput") for b in range(BPC)]
    out_d = [nc.dram_tensor(f"out{b}", [N, F], F32, kind="ExternalOutput") for b in range(BPC)]
    xd_pm = [nc.dram_tensor(f"xdpm{b}", [N, 132], F32) for b in range(BPC)]
    gg_dr = [nc.dram_tensor(f"ggdr{b}", [100, 2 * V], BF16) for b in range(BPC)]

    cd = {k: nc.inline_tensor(v, name=k) for k, v in consts.items()}
    with TileContext(nc) as tc:
        with tc.tile_pool(name="const", bufs=1) as cp:
            ct = {}
            for k, v in consts.items():
                t = cp.tile(list(v.shape), mybir.dt.from_np(v.dtype), tag=k, name=k)
                nc.sync.dma_start(out=t[:], in_=cd[k][:])
                ct[k] = t
            for b in range(BPC):
                build_batch(nc, tc, ct, xT_d[b], xb_d[b], msk_d[b], out_d[b],
                            xd_pm[b], gg_dr[b])
    nc.finalize()
    return nc


def make_consts(w_d1, b_d1, w_d2, b_d2, rot, wt0, bt0, wh0, th0, wt1, bt1, wh1, th1):
    rot50 = rot[:, :NB // 2]
    WR = (w_d2 @ rot50).astype(np.float32)
    # phase A produces h1' = elu+1; fold the -1 into downstream biases
    b2_eff = (b_d2 - w_d2.sum(0)).astype(np.float32)
    BRR = (b2_eff @ rot50).astype(np.float32)[None]

    jj = np.arange(50, dtype=np.float32)
    ACST = (BIG - jj)[None].repeat(128, 0).astype(np.float32)
    BCST = (BIG - 50.0 - jj)[None].repeat(128, 0).astype(np.float32)
    IOTAV = np.arange(V, dtype=np.float32)[None].repeat(128, 0).astype(bf)
    UT128 = (np.arange(128)[:, None] < np.arange(128)[None, :]).astype(bf)
    ut = (np.arange(NB)[:, None] < np.arange(NB)[None, :]).astype(bf)  # [t, tt]=1 iff t<tt
    UTREP = np.broadcast_to(ut[None], (128, NB, NB)).copy()

    W1 = w_d1.reshape(2, 128, 256).transpose(1, 0, 2).astype(np.float32).copy()
    B1 = b_d1.reshape(2, 128).T.astype(np.float32).copy()
    W2 = w_d2.reshape(2, 128, 128).transpose(1, 0, 2).astype(bf).copy()
    B2M = b2_eff[None].repeat(128, 0).astype(np.float32)
    WRc = WR.reshape(2, 128, 50).transpose(1, 0, 2).astype(np.float32).copy()

    # WCV[:, k, l, 0:256]=wt_l, 256:512=wh_l, 512:768=th_l  (k-split rows)
    wcv = np.zeros((128, 2, 2, 768), dtype=bf)
    for l, (wt, wh, th) in enumerate([(wt0, wh0, th0), (wt1, wh1, th1)]):
        for i, w in enumerate([wt, wh, th]):
            wc = w.reshape(2, 128, 256)
            wcv[:, 0, l, 256 * i:256 * (i + 1)] = wc[0].astype(bf)
            wcv[:, 1, l, 256 * i:256 * (i + 1)] = wc[1].astype(bf)
    # bias rows: [bt_l | zeros]  (wt path gets bias, wh path none)
    btl = np.zeros((1, 2, 512), dtype=bf)
    btl[0, 0, 0:256] = bt0.astype(bf)
    btl[0, 1, 0:256] = bt1.astype(bf)

    iotarow_i = np.arange(128, dtype=np.int32)[None].repeat(128, 0)
    iota100_i = np.arange(100, dtype=np.int32)[None].repeat(128, 0)
    pidf = (np.arange(NT, dtype=np.float32)[None, :] * 128
            + np.arange(128, dtype=np.float32)[:, None]).astype(np.float32)

    return {
        "W1": W1, "B1": B1, "W2": W2, "B2M": B2M, "WR": WRc, "BRR": BRR,
        "ACST": ACST, "BCST": BCST, "IOTAV": IOTAV, "UT128": UT128, "UTREP": UTREP,
        "ONES1_BF": np.ones((1, 128), dtype=bf),
        "ONES1_F": np.ones((1, 128), dtype=np.float32),
        "ONESC_BF": np.ones((128, 1), dtype=bf),
        "ONES100_F": np.ones((1, 100), dtype=np.float32),
        "IDENT": np.eye(128, dtype=np.float32),
        "IDENTB": np.eye(128, dtype=bf),
        "ID100": np.eye(100, dtype=np.float32),
        "E6": np.full((128, 1), 1e-6, dtype=np.float32),
        "WCV": wcv, "BTL": btl,
        "IOTAROW_I": iotarow_i, "IOTA100_I": iota100_i, "PIDF": pidf,
    }


_cache = {}


def kernel(x, msk, rot, w_d1, b_d1, w_d2, b_d2, wt0, bt0, wh0, th0, wt1, bt1, wh1, th1):
    x = np.asarray(x, np.float32)
    msk_u8 = np.asarray(msk).astype(np.uint8)
    consts = make_consts(np.asarray(w_d1, np.float32), np.asarray(b_d1, np.float32),
                         np.asarray(w_d2, np.float32), np.asarray(b_d2, np.float32),
                         np.asarray(rot, np.float32),
                         np.asarray(wt0, np.float32), np.asarray(bt0, np.float32),
                         np.asarray(wh0, np.float32), np.asarray(th0, np.float32),
                         np.asarray(wt1, np.float32), np.asarray(bt1, np.float32),
                         np.asarray(wh1, np.float32), np.asarray(th1, np.float32))
    if "nc" not in _cache:
        _cache["nc"] = build_nc(consts)
    nc = _cache["nc"]

    xT = np.ascontiguousarray(x.transpose(0, 2, 1))
    xb16 = np.ascontiguousarray(x.astype(bf))

    in_maps = []
    for c in range(NCORES):
        m = {}
        for b in range(BPC):
            gb = c * BPC + b
            m[f"xT{b}"] = xT[gb]
            m[f"xb{b}"] = xb16[gb]
            m[f"msk{b}"] = msk_u8[gb]
        in_maps.append(m)

    import os as _os
    _trace = _os.environ.get("KERNEL_TRACE", "") == "1"
    res = run_bass_kernel_spmd(nc, in_maps, core_ids=list(range(NCORES)), trace=_trace)
    if _trace:
        print(f"HW exec time: {res.exec_time_ns} ns")
        _cache["exec_time_ns"] = res.exec_time_ns
        _cache["res"] = res
    out = np.empty((B, N, F), np.float32)
    for c in range(NCORES):
        for b in range(BPC):
            out[c * BPC + b] = res.results[c][f"out{b}"]
    return out
